# revision 32
# baseline (speedup 1.0000x reference)
"""Trainium2 Bass kernel for masked causal dense attention.

Problem: B=8, Tq=Tv=2048, D=512 fp32.
  scores = q @ v^T; mask = v_mask & causal; scores -= 1e9*(~mask)
  out = softmax(scores) @ v; out *= q_mask

Sharding: data-parallel over batch, one batch element per NeuronCore (8 cores).

Per-core structure (flash-style, causal):
  for each 128-row q block b (v range W = 128*(b+1)):
    S = Q_b @ V^T          PE, K=512 accumulated in 128-chunks into PSUM,
                           plus a K=1 accumulation row that adds the -1e9
                           v_mask penalty vector (mask applied on PE for free)
    tri-mask diag block    DVE adds an upper-triangular -1e9 [128,128] const
    row max                DVE reduce_max per PSUM chunk + combine (negated)
    P = exp(S - max)       scalar engine activation, fused row-sum accumulate
    P^T                    PE transpose per 128-col block, PSUM -> SBUF copies
                           alternating DVE/ACT to balance engine load
    O += P^T.T @ V         PE, accumulated over v blocks in one PSUM bank
    out = O * qmask/l      DVE per-partition scale, DMA out
  The S/softmax stage of block b is emitted before the transpose/PV stage of
  block b-1, so PE never stalls waiting for the softmax chain.

Matmul dtype modes (ATTN_S_DTYPE / ATTN_O_DTYPE env, default 3pass/3pass):
  f32   exact, 4 cyc/row on the PE (slow)
  f32r  tf32-like, 1 cyc/row at width >= 256; avoid as a stationary operand
        (no fast-weight-load for 4-byte dtypes; ~50us of unhidden LDWEIGHTS)
  f16   fp16, 1 cyc/row, FWL-fast weight loads; ~2^-11 operand rounding
  3pass fp16 hi/lo split (host-side for Q/V^T/V, on-device for P), 3 matmul
        terms per contraction chunk: near-fp32 accuracy at 3x the cost
Measured on HW (8 cores, in-NEFF loop slope): 3pass/3pass ~224us rel 3.1e-5
(default); 3pass/f16 ~187us rel 3.9e-4; f32r/f32r ~104-130us rel 4.3e-3.
The v-mask add and the P hi/lo split both run off the PE (DVE/ACT have
headroom); P is transposed once in fp32 and split after the transpose; the
PV accumulation runs all hi-stream terms before lo-stream terms so the
in-order PE never waits on the DVE lo-subtract mid-group (-25us on HW).
"""

import os
import sys

import numpy as np

for _p in ("/opt/trn_rl_repo", "/root/.axon_site/_ro/trn_rl_repo"):
    if os.path.isdir(_p) and _p not in sys.path:
        sys.path.insert(0, _p)

import concourse.bacc as bacc
import concourse.bass as bass
import concourse.mybir as mybir
import concourse.tile as tile
from concourse.bass_utils import run_bass_kernel_spmd

B, Tq, Tv, D = 8, 2048, 2048, 512
P = 128
NB = Tq // P      # q blocks
ND = D // P       # contraction chunks for the S matmul
NVB = Tv // P     # v blocks
NEG = 1.0e9
F32 = mybir.dt.float32
F32R = mybir.dt.float32r

S_DTYPE = os.environ.get("ATTN_S_DTYPE", "f16")
O_DTYPE = os.environ.get("ATTN_O_DTYPE", "f16")
# Mask/row-max stage mode:
#   split (default): DVE adds the -1e9 v_mask penalty PSUM->SBUF, Pool (idle
#          otherwise; no PSUM port, SBUF is fine) does the per-chunk row-max
#          and the negated combine, ACT exp reads SBUF. Halves the DVE serial
#          chain vs doing add+rmax both on DVE, and frees the PSUM chunk
#          right after the add.
#   dve:   add + rmax + combine all on DVE, in-place in PSUM (original).
#   pe:    v_mask penalty added by a K=1 matmul row at the end of each S
#          accumulation group (costs W cycles/block on the PE but removes
#          the W-wide DVE add from the softmax critical chain; DVE then
#          only does rmax + combine).
#   tmr:   fused DVE tensor_mask_reduce -- fails at runtime on this
#          terminal (custom-opcode ucode not present); kept for reference.
# split (DVE add + Pool rmax) is impossible: gpsimd tensor_reduce is
# partition-axis only, and Pool has no PSUM port.
MASK_MODE = os.environ.get("ATTN_MASK_MODE", "pe")
USE_TMR = MASK_MODE == "tmr"
USE_SPLIT = MASK_MODE == "split"
USE_PE_MASK = MASK_MODE == "pe"
# DMA-xbar transpose for P^T (2-byte o_dt only; not the 3pass fp32 path).
# Measured worse than PE transposes: the per-instruction HWDGE cadence
# (625ns gen + 650ns start latency) starves the PV stream. Default off.
USE_DMAT = (int(os.environ.get("ATTN_DMAT", "0"))
            and O_DTYPE in ("f16",))
# v_len >= Tv//2 by the input spec, so score chunks that end at or below
# VMIN never contain a masked column -- their penalty row is skipped.
VMIN = Tv // 2


def _mm_dt(name):
    return F32R if name == "f32r" else F32


def _chunk_widths(W):
    """Split W (multiple of 128) into PSUM-bank chunks <= 512 wide, avoiding
    128-wide chunks (f32r matmuls need width >= 256 for full PE rate)."""
    ws = []
    rem = W
    while rem > 512:
        ws.append(512)
        rem -= 512
    if rem == 128 and ws:
        ws[-1] = 384
        ws.append(256)
    else:
        ws.append(rem)
    return ws


def _chunk_spans(b):
    """[(v0, w)] chunk column spans for q block b."""
    spans = []
    v0 = 0
    for w in _chunk_widths((b + 1) * P):
        spans.append((v0, w))
        v0 += w
    return spans


CHUNK_BASE = []
NCHT = 0
for _b in range(NB):
    CHUNK_BASE.append(NCHT)
    NCHT += len(_chunk_spans(_b))


def build_nc(s_dtype=None, o_dtype=None, loop_n=None):
    """Build + compile the SPMD module. loop_n: wrap the per-block body in a
    hardware loop with Internal DRAM tensors (timing mode, no host I/O).

    s_dtype / o_dtype: "f32" | "f32r" | "3pass". 3pass = fp16 hi/lo split
    (host-side for Q/V^T/V, on-device for P), 3 matmul terms per contraction
    chunk -- near-fp32 accuracy at 3x the f32r matmul cost."""
    s_mode = s_dtype or S_DTYPE
    o_mode = o_dtype or O_DTYPE
    timing = loop_n is not None
    kin = "Internal" if timing else "ExternalInput"
    kout = "Internal" if timing else "ExternalOutput"
    F16 = mybir.dt.float16

    nc = bacc.Bacc("TRN2", target_bir_lowering=False, num_devices=B)
    if s_mode == "3pass":
        s_dt = F16
        qts = [nc.dram_tensor(n, [D, Tq], F16, kind=kin)
               for n in ("qt_hi", "qt_lo")]
        vts = [nc.dram_tensor(n, [D, Tv], F16, kind=kin)
               for n in ("vt_hi", "vt_lo")]
        terms = [(0, 0), (0, 1), (1, 0)]   # (qt stream, vt stream)
    else:
        s_dt = F16 if s_mode == "f16" else _mm_dt(s_mode)
        qts = [nc.dram_tensor("qt", [D, Tq], s_dt, kind=kin)]
        vts = [nc.dram_tensor("vt", [D, Tv], s_dt, kind=kin)]
        terms = [(0, 0)]
    if o_mode == "3pass":
        # P is split on device into fp16 hi/lo; V is split on host.
        o_dt = F16            # dtype of P^T tiles / identity / V streams
        p_dt = F32            # exp output stays full precision for the split
        vs = [nc.dram_tensor(n, [Tv, D], F16, kind=kin)
              for n in ("v_hi", "v_lo")]
        oterms = [(0, 0), (0, 1), (1, 0)]  # (pt stream, v stream)
    else:
        o_dt = F16 if o_mode == "f16" else _mm_dt(o_mode)
        p_dt = o_dt
        vs = [nc.dram_tensor("v", [Tv, D], o_dt, kind=kin)]
        oterms = [(0, 0)]
    if USE_TMR:
        # per-(block, chunk) prefix-mask end columns, [128 q, NCHT] fp32:
        # mend[q, CHUNK_BASE[b]+c] = clip(min(128*b+q+1, v_len) - v0_c, 0, w_c)
        mend = nc.dram_tensor("mend", [P, NCHT], F32, kind=kin)
    elif USE_PE_MASK:
        # penalty row consumed by the K=1 matmul term; matches the S matmul
        # dtype (f16 cannot hold -1e9; any value <= -1000 zeroes the exp)
        negv = nc.dram_tensor("negv", [1, Tv], s_dt, kind=kin)
        mend = None
    else:
        negv = nc.dram_tensor("negv", [1, Tv], F32, kind=kin)
        mend = None
    qsc = nc.dram_tensor("qsc", [Tq], F32, kind=kin)
    out = nc.dram_tensor("out", [Tq, D], F32, kind=kout)
    if timing:
        tick_in = nc.dram_tensor("tick_in", [1, 1], F32, kind="ExternalInput")
        tick_out = nc.dram_tensor("tick_out", [1, 1], F32, kind="ExternalOutput")

    from contextlib import ExitStack

    with tile.TileContext(nc) as tc, ExitStack() as ctx:
        const = ctx.enter_context(tc.tile_pool(name="const", bufs=1))
        big = ctx.enter_context(tc.tile_pool(name="big", bufs=1))
        qtp = ctx.enter_context(tc.tile_pool(name="qtp", bufs=3))
        pp = ctx.enter_context(tc.tile_pool(name="pp", bufs=3))
        ptp = ctx.enter_context(tc.tile_pool(name="ptp", bufs=2))
        outp = ctx.enter_context(tc.tile_pool(name="outp", bufs=3))
        smallp = ctx.enter_context(tc.tile_pool(name="smallp", bufs=3))
        if USE_TMR or USE_SPLIT:
            # masked scores land in SBUF (frees the PSUM chunk right after
            # the DVE mask/add pass, and lets Pool do the row-max)
            smp = ctx.enter_context(tc.tile_pool(name="smp", bufs=3))
        sbuf_scores = USE_TMR or USE_SPLIT
        sps = ctx.enter_context(
            tc.tile_pool(name="sps", bufs=4 if sbuf_scores else 5, space="PSUM"))
        ops = ctx.enter_context(tc.tile_pool(
            name="ops", bufs=2 if (sbuf_scores or USE_DMAT) else 1,
            space="PSUM"))
        # with DMA transposes pts only serves the warmup matmuls
        pts = ctx.enter_context(tc.tile_pool(
            name="pts", bufs=1 if USE_DMAT else 2, space="PSUM"))

        # --- constants ---
        ident32 = const.tile([P, P], F32)
        nc.gpsimd.memset(ident32, 0.0)
        nc.gpsimd.affine_select(
            out=ident32, in_=ident32, compare_op=mybir.AluOpType.not_equal,
            fill=1.0, base=0, pattern=[[-1, P]], channel_multiplier=1,
        )
        if o_dt == F32:
            ident = ident32
        else:
            ident = const.tile([P, P], o_dt)
            nc.vector.tensor_copy(ident, ident32)
        if USE_PE_MASK:
            ones1 = const.tile([1, P], s_dt)
            nc.gpsimd.memset(ones1, 1.0)
        # tri[q, v] = -NEG where v > q else 0 (within-diagonal-block causal)
        tri = const.tile([P, P], F32)
        nc.gpsimd.memset(tri, 0.0)
        nc.gpsimd.affine_select(
            out=tri, in_=tri, compare_op=mybir.AluOpType.is_ge,
            fill=-NEG, base=0, pattern=[[-1, P]], channel_multiplier=1,
        )

        def emit_prelude():
            if USE_TMR:
                negv_b = big.tile([P, NCHT], F32, tag="mendsb")
                nc.sync.dma_start(out=negv_b, in_=mend.ap())
            elif USE_PE_MASK:
                negv_b = big.tile([1, Tv], s_dt, tag="negvb")
                nc.sync.dma_start(out=negv_b, in_=negv.ap())
            else:
                negv_b = big.tile([P, Tv], F32, tag="negvb")
                nc.sync.dma_start(
                    out=negv_b, in_=negv.ap()[0].partition_broadcast(P)
                )
            qsc_sb = big.tile([P, NB], F32, tag="qscsb")
            nc.sync.dma_start(
                out=qsc_sb, in_=qsc.ap().rearrange("(b p) -> p b", p=P)
            )
            vt_sbs = [big.tile([P, ND, Tv], s_dt, tag=f"vtsb{i}",
                                name=f"vtsb{i}") for i in range(len(vts))]
            qt_sbs = [big.tile([P, ND, Tq], s_dt, tag=f"qtsb{i}",
                                name=f"qtsb{i}") for i in range(len(qts))]
            v_sbs = [big.tile([P, NVB, D], o_dt, tag=f"vsb{i}",
                              name=f"vsb{i}") for i in range(len(vs))]
            # DMA in column-range groups so the first q blocks' operands land
            # early and the PE doesn't stall on the full 16MB prelude. The
            # first two groups' qt loads ride the (still idle) ACT DMA queue
            # in parallel with vt on the sync queue; later groups stay off
            # the ACT queue so they can't delay the exp chain.
            groups = [(s, 512) for s in range(0, Tv, 512)]
            for gi, (s0, G) in enumerate(groups):
                qt_q = nc.scalar if gi < 2 else nc.sync
                for c in range(ND):
                    for vt, vt_sb in zip(vts, vt_sbs):
                        nc.sync.dma_start(
                            out=vt_sb[:, c, s0:s0 + G],
                            in_=vt[c * P:(c + 1) * P, s0:s0 + G],
                        )
                for c in range(ND):
                    for qt, qt_sb in zip(qts, qt_sbs):
                        qt_q.dma_start(
                            out=qt_sb[:, c, s0:s0 + G],
                            in_=qt[c * P:(c + 1) * P, s0:s0 + G],
                        )
                for j in range(s0 // P, (s0 + G) // P):
                    for v, v_sb in zip(vs, v_sbs):
                        nc.sync.dma_start(
                            out=v_sb[:, j, :], in_=v[j * P:(j + 1) * P, :]
                        )
            return negv_b, qsc_sb, vt_sbs, v_sbs, qt_sbs

        def emit_softmax_block(b, negv_b, vt_sbs, qt_sbs):
            """S matmuls + masked softmax for q block b."""
            W = (b + 1) * P
            widths = _chunk_widths(W)
            nch = len(widths)

            p_sb = pp.tile([P, W], p_dt, tag="p")
            colmax = smallp.tile([P, 4], F32, tag="colmax")
            lsum = smallp.tile([P, 4], F32, tag="lsum")
            negm = smallp.tile([P, 1], F32, tag="negm")
            if USE_TMR or USE_SPLIT:
                s_m = smp.tile([P, W], F32, tag="sm", name="sm")
            else:
                s_m = None
            s_tiles = []
            v0 = 0
            for c, w in enumerate(widths):
                s_t = sps.tile([P, 512], F32, tag="s")
                # chunks fully below VMIN never contain a masked column
                # (v_len >= VMIN by the input spec): skip their penalty row
                pen = USE_PE_MASK and v0 + w > VMIN
                n_mm = ND * len(terms) + (1 if pen else 0)
                mi = 0
                for dc in range(ND):
                    for qi, vi in terms:
                        nc.tensor.matmul(
                            s_t[:, :w],
                            qt_sbs[qi][:, dc, b * P:(b + 1) * P],
                            vt_sbs[vi][:, dc, v0:v0 + w],
                            start=(mi == 0),
                            stop=(mi == n_mm - 1),
                        )
                        mi += 1
                if pen:
                    # K=1 accumulation row adds the v_mask penalty on the PE
                    nc.tensor.matmul(
                        s_t[:, :w],
                        ones1,
                        negv_b[:, v0:v0 + w],
                        start=False,
                        stop=True,
                    )
                if USE_PE_MASK:
                    if c == nch - 1:
                        nc.vector.tensor_add(
                            out=s_t[:, w - P:w], in0=s_t[:, w - P:w], in1=tri
                        )
                    nc.vector.reduce_max(
                        out=colmax[:, c:c + 1], in_=s_t[:, :w],
                        axis=mybir.AxisListType.X,
                    )
                    s_tiles.append((s_t[:, :w], v0, w))
                elif USE_SPLIT:
                    if c == nch - 1:
                        nc.vector.tensor_add(
                            out=s_t[:, w - P:w], in0=s_t[:, w - P:w], in1=tri
                        )
                    nc.vector.tensor_add(
                        out=s_m[:, v0:v0 + w], in0=s_t[:, :w],
                        in1=negv_b[:, v0:v0 + w],
                    )
                    nc.gpsimd.reduce_max(
                        out=colmax[:, c:c + 1], in_=s_m[:, v0:v0 + w],
                        axis=mybir.AxisListType.X,
                    )
                    s_tiles.append((s_m[:, v0:v0 + w], v0, w))
                elif USE_TMR:
                    # one DVE pass: causal+v_len prefix mask (-> -FLT_MAX)
                    # plus cascaded row-max; the last chunk writes -max.
                    last = c == nch - 1
                    nc.vector.tensor_mask_reduce(
                        out=s_m[:, v0:v0 + w], in_=s_t[:, :w],
                        mask_start=0.0,
                        mask_end=negv_b[:, CHUNK_BASE[b] + c:CHUNK_BASE[b] + c + 1],
                        scale=1.0,
                        accum_in=(-3.0e38 if c == 0 else colmax[:, c - 1:c]),
                        op=mybir.AluOpType.max,
                        negate_accum=last,
                        accum_out=(negm if last else colmax[:, c:c + 1]),
                    )
                    s_tiles.append((s_m[:, v0:v0 + w], v0, w))
                else:
                    if c == nch - 1:
                        nc.vector.tensor_add(
                            out=s_t[:, w - P:w], in0=s_t[:, w - P:w], in1=tri
                        )
                    if v0 + w > VMIN:
                        nc.vector.tensor_add(
                            out=s_t[:, :w], in0=s_t[:, :w],
                            in1=negv_b[:, v0:v0 + w],
                        )
                    nc.vector.reduce_max(
                        out=colmax[:, c:c + 1], in_=s_t[:, :w],
                        axis=mybir.AxisListType.X,
                    )
                    s_tiles.append((s_t[:, :w], v0, w))
                v0 += w
            if USE_SPLIT:
                nc.gpsimd.tensor_reduce(
                    out=negm, in_=colmax[:, :nch], axis=mybir.AxisListType.X,
                    op=mybir.AluOpType.max, negate=True,
                )
            elif not USE_TMR:
                nc.vector.tensor_reduce(
                    out=negm, in_=colmax[:, :nch], axis=mybir.AxisListType.X,
                    op=mybir.AluOpType.max, negate=True,
                )
            for c, (src, v0, w) in enumerate(s_tiles):
                nc.scalar.activation(
                    out=p_sb[:, v0:v0 + w], in_=src,
                    func=mybir.ActivationFunctionType.Exp,
                    bias=negm, scale=1.0,
                    accum_out=lsum[:, c:c + 1],
                )
            l = smallp.tile([P, 1], F32, tag="l")
            nc.vector.tensor_reduce(
                out=l, in_=lsum[:, :nch], axis=mybir.AxisListType.X,
                op=mybir.AluOpType.add,
            )
            linv = smallp.tile([P, 1], F32, tag="linv")
            nc.vector.reciprocal(out=linv, in_=l)
            return p_sb, linv, W

        def emit_pv_block(b, p_sb, linv, W, qsc_sb, v_sbs):
            """Transpose P and accumulate O = P^T.T @ V for q block b."""
            nvb = W // P
            if o_mode == "3pass":
                # transpose the fp32 P once (2 cyc/row = same PE cycles as two
                # fp16 transposes, half the instructions), then split into
                # fp16 hi/lo in the [v,q] domain straight off the PSUM tile:
                # hi = rounding copy (ACT), lo = residual subtract (DVE)
                pt_hi = ptp.tile([P, W], F16, tag="pt0", name="pt0")
                pt_lo = ptp.tile([P, W], F16, tag="pt1", name="pt1")
                for g in range(0, nvb, 4):
                    gn = min(4, nvb - g)
                    pt_ps = pts.tile([P, 512], F32, tag="ptps", name="ptps")
                    for k in range(gn):
                        j = g + k
                        nc.tensor.transpose(
                            out=pt_ps[:, k * P:(k + 1) * P],
                            in_=p_sb[:, j * P:(j + 1) * P],
                            identity=ident32,
                        )
                    nc.scalar.copy(
                        pt_hi[:, g * P:(g + gn) * P], pt_ps[:, :gn * P]
                    )
                    nc.vector.tensor_sub(
                        out=pt_lo[:, g * P:(g + gn) * P],
                        in0=pt_ps[:, :gn * P],
                        in1=pt_hi[:, g * P:(g + gn) * P],
                    )
                pt_sbs = [pt_hi, pt_lo]
            elif USE_DMAT:
                # Hybrid P^T: early chunks ride the DMA xbar (their exp
                # output lands long before PV(b) consumes them), only the
                # final chunk is PE-transposed (its lead time is short and
                # the DMA start latency ~1.3us would stall the PE). All
                # transposes stay on the one SP queue -- the xbar must not
                # run concurrently with other SBUF->SBUF DMA.
                pt_sb = ptp.tile([P, W], o_dt, tag="pt0", name="pt0")
                pt_sbs = [pt_sb]
                spans = _chunk_spans(b)
                for v0, w in spans[1:]:
                    nc.sync.dma_start(
                        out=pt_sb[:, v0:v0 + w].rearrange(
                            "p (j q) -> p j q", q=P),
                        in_=p_sb[:, v0:v0 + w],
                        transpose=True,
                    )
                v0, w = spans[0]
                for g in range(v0 // P, (v0 + w) // P, 4):
                    gn = min(4, (v0 + w) // P - g)
                    pt_ps = pts.tile([P, 512], o_dt, tag="ptps", name="ptps")
                    for k in range(gn):
                        j = g + k
                        nc.tensor.transpose(
                            out=pt_ps[:, k * P:(k + 1) * P],
                            in_=p_sb[:, j * P:(j + 1) * P],
                            identity=ident,
                        )
                    nc.vector.tensor_copy(
                        pt_sb[:, g * P:(g + gn) * P], pt_ps[:, :gn * P]
                    )
            else:
                pt_sb = ptp.tile([P, W], o_dt, tag="pt0", name="pt0")
                pt_sbs = [pt_sb]
                for g in range(0, nvb, 4):
                    gn = min(4, nvb - g)
                    pt_ps = pts.tile([P, 512], o_dt, tag="ptps", name="ptps")
                    for k in range(gn):
                        j = g + k
                        nc.tensor.transpose(
                            out=pt_ps[:, k * P:(k + 1) * P],
                            in_=p_sb[:, j * P:(j + 1) * P],
                            identity=ident,
                        )
                    # balance the PSUM->SBUF copies between DVE and ACT
                    if (g // 4) % 3 == 2:
                        nc.scalar.copy(
                            pt_sb[:, g * P:(g + gn) * P], pt_ps[:, :gn * P]
                        )
                    else:
                        nc.vector.tensor_copy(
                            pt_sb[:, g * P:(g + gn) * P], pt_ps[:, :gn * P]
                        )
            o_ps = ops.tile([P, D], F32, tag="o")
            # hi-stream terms first, lo-stream terms last: the lo tiles come
            # off a DVE subtract, so deferring them keeps the in-order PE from
            # stalling mid-accumulation if DVE lags.
            seq = ([(j, pi, vi) for j in range(nvb)
                    for pi, vi in oterms if pi == 0] +
                   [(j, pi, vi) for j in range(nvb)
                    for pi, vi in oterms if pi != 0])
            for mi, (j, pi, vi) in enumerate(seq):
                nc.tensor.matmul(
                    o_ps,
                    pt_sbs[pi][:, j * P:(j + 1) * P],
                    v_sbs[vi][:, j, :],
                    start=(mi == 0),
                    stop=(mi == len(seq) - 1),
                )
            fs = smallp.tile([P, 1], F32, tag="fs")
            nc.vector.tensor_mul(fs, linv, qsc_sb[:, b:b + 1])
            o_sb = outp.tile([P, D], F32, tag="osb")
            # per-partition qmask/l scale on ACT (DVE is the busier engine)
            nc.scalar.activation(
                out=o_sb, in_=o_ps,
                func=mybir.ActivationFunctionType.Identity,
                bias=0.0, scale=fs,
            )
            nc.gpsimd.dma_start(out=out[b * P:(b + 1) * P, :], in_=o_sb)

        def emit_warmup():
            """Dummy matmuls on constant tiles while the prelude DMA streams:
            keeps the PE busy through the HAM activity window so the real
            matmuls start at full clock instead of the cold half-rate."""
            warm_ps = pts.tile([P, P], F32, tag="ptps", name="warm_ps")
            warm16 = const.tile([P, P], F16)
            nc.vector.tensor_copy(warm16, ident32)
            for _ in range(100):   # ~5us of PE warmup at 1 cyc/row
                nc.tensor.matmul(warm_ps, warm16, warm16,
                                 start=True, stop=True)

        def emit_body(preloaded):
            negv_row, qsc_sb, vt_sb, v_sb, qt_sb = preloaded
            # Block order 1..15 then 0: the schedule's tail PV is the tiny
            # block 0, so the last softmax chain hides under PV(15).
            # Early (small) blocks use a 2-deep softmax->PV pipeline so short
            # S stages still cover the previous block's softmax latency.
            # 1..15 then 0: the schedule tail is the tiny block 0, so the
            # last real softmax chain (15) hides under PV(15) and the loop
            # boundary carries no pipeline bubble into the next iteration.
            order = list(range(1, NB)) + [0]
            nch_of = lambda b: len(_chunk_widths((b + 1) * P))
            pending = []
            for i, b in enumerate(order):
                cur = emit_softmax_block(b, negv_row, vt_sb, qt_sb)
                pending.append((b, cur))
                # lag-2 (two softmax blocks in flight) while both blocks' S
                # chunks fit in the sps PSUM pool; the small blocks' fixed
                # sem-chain latency then hides under two S stages of cover.
                if i + 1 < len(order):
                    lag2_max = int(os.environ.get("ATTN_LAG2_MAX", "8"))
                    lag = 2 if (int(os.environ.get("ATTN_LAG2", "1")) and
                                nch_of(b) + nch_of(order[i + 1]) <= lag2_max) else 1
                else:
                    lag = 1
                while len(pending) > lag:
                    bb, cc = pending.pop(0)
                    emit_pv_block(bb, *cc, qsc_sb, v_sb)
            while pending:
                bb, cc = pending.pop(0)
                emit_pv_block(bb, *cc, qsc_sb, v_sb)

        if timing:
            tick = const.tile([1, 1], F32)
            nc.sync.dma_start(out=tick, in_=tick_in[:, :])
            preloaded = emit_prelude()
            emit_warmup()
            with tc.For_i(0, loop_n, 1,
                          staggered_reset=bool(int(
                              os.environ.get("ATTN_STAGGER", "1")))):
                emit_body(preloaded)
            nc.sync.dma_start(out=tick_out[:, :], in_=tick)
        else:
            preloaded = emit_prelude()
            emit_warmup()
            emit_body(preloaded)

    nc.compile()
    return nc


_NC_CACHE = {}


def _get_nc():
    key = (S_DTYPE, O_DTYPE)
    if key not in _NC_CACHE:
        _NC_CACHE[key] = build_nc()
    return _NC_CACHE[key]


def _f16_split(x):
    hi = x.astype(np.float16)
    lo = (x - hi.astype(np.float32)).astype(np.float16)
    return hi, lo


def make_in_maps(query, value, q_mask, v_mask, s_mode=None, o_mode=None):
    s_mode = s_mode or S_DTYPE
    o_mode = o_mode or O_DTYPE
    in_maps = []
    for b in range(B):
        q = np.asarray(query[b], dtype=np.float32)
        val = np.asarray(value[b], dtype=np.float32)
        m = {"qsc": np.asarray(q_mask[b], dtype=np.float32)}
        if o_mode == "3pass":
            vc = np.ascontiguousarray(val)
            m["v_hi"], m["v_lo"] = _f16_split(vc)
        elif o_mode == "f16":
            m["v"] = np.ascontiguousarray(val).astype(np.float16)
        else:
            m["v"] = np.ascontiguousarray(val)
        if USE_TMR:
            # causal & length-mask as per-(block, chunk) prefix ends
            v_len = int(np.asarray(v_mask[b]).sum())
            mend = np.zeros((P, NCHT), np.float32)
            qv = np.arange(P)
            for blk in range(NB):
                for c, (v0, w) in enumerate(_chunk_spans(blk)):
                    e = np.minimum(blk * P + qv + 1, v_len) - v0
                    mend[:, CHUNK_BASE[blk] + c] = np.clip(e, 0, w)
            m["mend"] = mend
        elif USE_PE_MASK:
            pen = -60000.0 if s_mode in ("f16", "3pass") else -NEG
            ndt = np.float16 if s_mode in ("f16", "3pass") else np.float32
            m["negv"] = np.where(v_mask[b], 0.0, pen).astype(ndt)[None, :]
        else:
            m["negv"] = np.where(
                v_mask[b], 0.0, -NEG).astype(np.float32)[None, :]
        if s_mode == "3pass":
            qt = np.ascontiguousarray(q.T)
            vt = np.ascontiguousarray(val.T)
            m["qt_hi"], m["qt_lo"] = _f16_split(qt)
            m["vt_hi"], m["vt_lo"] = _f16_split(vt)
        elif s_mode == "f16":
            m["qt"] = np.ascontiguousarray(q.T).astype(np.float16)
            m["vt"] = np.ascontiguousarray(val.T).astype(np.float16)
        else:
            m["qt"] = np.ascontiguousarray(q.T)
            m["vt"] = np.ascontiguousarray(val.T)
        in_maps.append(m)
    return in_maps


def kernel(query, value, q_mask, v_mask, **kw):
    nc = _get_nc()
    in_maps = make_in_maps(query, value, q_mask, v_mask)
    res = run_bass_kernel_spmd(nc, in_maps, core_ids=list(range(B)))
    return np.stack([res.results[c]["out"] for c in range(B)], axis=0)

